# revision 29
# baseline (speedup 1.0000x reference)
"""Trainium2 Bass kernel for batched attention with LayerNorm'd projections.

Reference computation (per batch element b):
    keys    = LN(Y[b] @ K) * g1 + b1          [S, H]
    queries = LN(X[b] @ Q) * g2 + b2          [S, H]
    alpha   = softmax(queries @ keys.T / H)   [S, S]
    out[b]  = alpha @ Y[b]                    [S, F]

Shapes: B=8, S=2048, F=H=1024. Data-parallel: one batch element per
NeuronCore, 8 cores, no collectives. Measured ~242us on silicon at
~4.2e-3 relative error (prior q-major-logits version: ~257us).

Device pipeline per core (transposed-logits orientation):
  A: q/k projections in natural [S,H] layout from fp8 XT/YT (contraction
     F on partitions, DoubleRow). LayerNorm stats via bn_stats on PSUM
     with 3 rotating 2-bank PSUM sets. The q side applies full LN
     ((x-mu)*rstd, split DVE/ACT) because the logits matmul needs a
     zero-mean q operand; the k side skips the apply entirely — raw k is
     transposed and rstd_k is folded into the exp scale later, and the
     k-mean term cancels exactly against zero-mean q̂:
        logit[t,q] = rstd_k[t]/H * sum_h k_raw[h,t] * qhat[h,q]
     PE 128x128 transposes move q̂/k_raw into H-major fp8 qT/kT with
     3072B row stride (2048B pair stride on the *streamed* operand
     halves the DoubleRow rate).
  B: logits computed TRANSPOSED, lT[t,q] = kT_block^T @ qT, per t-stripe
     [128, 2048]. ACT applies exp with per-partition scale rstd_k[t]/H;
     DVE applies exp(l)-1 during the fp8 cast into expT8 [t, q] (delta
     softmax: values ~±0.2 quantize ~20x better than ~1.0). Because
     expT8 is already t-major, NO alpha transposes are needed for the
     value matmul (the old orientation burned 256 PE transposes here).
  C: U stripe [q,F] = expT8_slice^T @ Y_aug where Y_aug carries a ones
     column: the same accumulation group yields the softmax denominator
     column den[q]-S for free. Exact f32 colsum(Y) (host computed) is
     added by DVE; the mandatory PSUM->SBUF copy applies 1/den.

Hardware constraints that shaped this:
  - fp32 matmul is 4x slower than bf16; fp8 DoubleRow is ~2x faster.
    Everything matmul-facing is fp8 with f32 PSUM accumulation.
  - PSUM is 8 banks: phase A uses 3x2 (projections) + 2 (transposes);
    phases B/C use 3 rotating logits banks + 2x2 output + 1 denominator.
  - The PE engine queue is strict FIFO and the HAM clock gate re-throttles
    the PE to 1.2 GHz after idle gaps: phase-A transposes are emitted with
    a 2-stripe lag behind the projection matmuls, and the 3 logits banks
    rotate globally (not per-stripe), so the PE never waits on a vector-
    engine epilogue. Before these two fixes ~15% of matmuls ran cold.
  - The first ~6us are fixed framework preamble (semaphore init + engine
    table loads) and the first projection operands take ~12us to land
    (per-queue DMA rate ~25-30GB/s); ~64 throwaway identity transposes
    fill that window so the HAM gate is already warm at first real work.
  - Run-to-run variance: under sustained load the chip drops to ~2.0 GHz
    (P0 power state), inflating the whole kernel ~20%; compare timings
    via the warm p50 matmul duration (215ns at 2.4 GHz, 259ns at 2.0).
"""

import numpy as np
import ml_dtypes

import concourse.bass as bass
import concourse.bacc as bacc
import concourse.tile as tile
from concourse import mybir
from concourse.bass_utils import run_bass_kernel_spmd
from concourse.masks import make_identity

BF16 = mybir.dt.bfloat16
FP8 = mybir.dt.float8e4
F32 = mybir.dt.float32
AF = mybir.ActivationFunctionType

S = 2048  # sequence length per core
SP = 3072  # padded qT/kT row stride (odd multiple of 1KB: avoids SBUF bank conflicts in DoubleRow pair fetch)
F = 1024  # input feature dim
FA = F + 16  # Y row padded with a ones column at index F (denominator trick)
H = 1024  # hidden dim
P = 128  # partitions
NS = S // P  # 16 sequence stripes
NF = F // P  # 8 contraction tiles for projections
NH = H // P  # 8 hidden tiles
NC = 512  # matmul free-dim chunk (one PSUM bank)
EPS = 1e-5


def _build_nc(affine1: bool, affine2: bool) -> bass.Bass:
    nc = bacc.Bacc(None)

    # All operands host-packed so each SBUF partition's data is one
    # contiguous DRAM run (2KB+ descriptor lines; strided layouts move
    # ~1KB per ~67ns descriptor slot and halve effective DMA rate).
    xt = nc.declare_dram_parameter("XT", [P, NF * S], FP8, isOutput=False)[:]
    yt = nc.declare_dram_parameter("YT", [P, NF * S], FP8, isOutput=False)[:]
    y8 = nc.declare_dram_parameter("Y8", [P, NS * FA], FP8, isOutput=False)[:]
    cs = nc.declare_dram_parameter("CS", [P, F], F32, isOutput=False)[:]
    kw = nc.declare_dram_parameter("Kw", [P, NF * H], FP8, isOutput=False)[:]
    qw = nc.declare_dram_parameter("Qw", [P, NF * H], FP8, isOutput=False)[:]
    g1 = b1 = g2 = b2 = None
    if affine1:
        g1 = nc.declare_dram_parameter("g1r", [1, H], BF16, isOutput=False)[:]
        b1 = nc.declare_dram_parameter("b1r", [1, H], BF16, isOutput=False)[:]
    if affine2:
        g2 = nc.declare_dram_parameter("g2r", [1, H], BF16, isOutput=False)[:]
        b2 = nc.declare_dram_parameter("b2r", [1, H], BF16, isOutput=False)[:]
    out = nc.declare_dram_parameter("out", [S, F], F32, isOutput=True)[:]

    with tile.TileContext(nc) as tc:
        with (
            tc.tile_pool(name="persist", bufs=1) as persist,
            tc.tile_pool(name="stats", bufs=8) as stats_pool,
        ):
            # Persistent SBUF tensors (whole-kernel lifetime).
            # Per-partition: qT 24k + kT 24k + expT8 32k + y_sb 16.25k +
            # crep 4k + small = ~101k.
            qT = persist.tile([P, NH, SP], FP8, tag="qT")  # q̂T [H, S+pad]
            kT = persist.tile([P, NH, SP], FP8, tag="kT")  # k_rawT [H, S+pad]
            # expT8 rows padded to the 3072B stride: it is the value matmul's
            # stationary operand, and LDWEIGHTS DoubleRow pair fetch at a
            # 2048B pair stride measured ~14ns/MM slower than at 3072B.
            expT8 = persist.tile([P, NS, SP], FP8, tag="expT8")  # expm1T [Sk, Sq]
            recips = persist.tile([P, NS], F32, tag="recips")
            rstdk_h = persist.tile([P, NS], F32, tag="rstdk_h")  # rstd_k/H per t
            y_sb = persist.tile([P, NS, FA], FP8, tag="y_sb")  # Y_aug [Sk, F+1s]
            crep = persist.tile([P, F], F32, tag="crep")  # colsum(Y) bcast
            eps_sb = persist.tile([P, 1], F32, tag="eps")
            nc.vector.memset(eps_sb, EPS)
            eps_h2 = persist.tile([P, 1], F32, tag="eps_h2")
            nc.vector.memset(eps_h2, EPS * H * H)
            identb = persist.tile([P, P], BF16, tag="identb")
            make_identity(nc, identb)
            warmed = False

            # ---- Phase A: projections + LN + transpose to H-major ----
            DR = mybir.MatmulPerfMode.DoubleRow
            with (
                tc.tile_pool(name="operands", bufs=1) as operands,
                tc.tile_pool(name="work", bufs=4) as work,
                tc.tile_pool(name="psumA", bufs=1, space="PSUM") as psumA,
                tc.tile_pool(name="psumAT", bufs=2, space="PSUM") as psumAT,
            ):
                # All projection operands SBUF-resident in fp8:
                # xt/yt 16k + q/k 8k each = 48k per partition.
                xt_sb = operands.tile([P, NF, S], FP8, tag="xt_sb")
                yt_sb = operands.tile([P, NF, S], FP8, tag="yt_sb")
                q_sb = operands.tile([P, NF, H], FP8, tag="q_sb")
                k_sb = operands.tile([P, NF, H], FP8, tag="k_sb")
                xt_r = xt.rearrange("p (fb s) -> p fb s", fb=NF)
                yt_r = yt.rearrange("p (fb s) -> p fb s", fb=NF)
                qw_r = qw.rearrange("p (fb h) -> p fb h", fb=NF)
                kw_r = kw.rearrange("p (fb h) -> p fb h", fb=NF)
                # Load order tuned for earliest first-stripe start: the
                # first k-projection stripes need k pairs (full) and the
                # low column quarter of every yt pair. Whole k pairs are
                # single DMAs (contiguous 2KB/partition runs thanks to
                # the host packing); yt/xt are column-split since stripes
                # consume them 128 columns at a time.
                for i in range(NF // 2):
                    # Partition-halved DMAs keep the contiguous 2KB lines
                    # while landing each piece on its own queue, halving
                    # the time to the first stripe's operands.
                    for plo, phi in ((0, P // 2), (P // 2, P)):
                        nc.sync.dma_start(
                            out=k_sb[plo:phi, 2 * i : 2 * i + 2, :],
                            in_=kw_r[plo:phi, 2 * i : 2 * i + 2, :],
                        )
                        nc.sync.dma_start(
                            out=yt_sb[plo:phi, 2 * i : 2 * i + 2, 0 : S // 2],
                            in_=yt_r[plo:phi, 2 * i : 2 * i + 2, 0 : S // 2],
                        )
                for i in range(NF // 2):
                    nc.sync.dma_start(
                        out=yt_sb[:, 2 * i : 2 * i + 2, S // 2 : S],
                        in_=yt_r[:, 2 * i : 2 * i + 2, S // 2 : S],
                    )
                for i in range(NF // 2):
                    nc.sync.dma_start(
                        out=q_sb[:, 2 * i : 2 * i + 2, :],
                        in_=qw_r[:, 2 * i : 2 * i + 2, :],
                    )
                    nc.sync.dma_start(
                        out=xt_sb[:, 2 * i : 2 * i + 2, 0 : S // 2],
                        in_=xt_r[:, 2 * i : 2 * i + 2, 0 : S // 2],
                    )
                for i in range(NF // 2):
                    nc.sync.dma_start(
                        out=xt_sb[:, 2 * i : 2 * i + 2, S // 2 : S],
                        in_=xt_r[:, 2 * i : 2 * i + 2, S // 2 : S],
                    )
                if not warmed:
                    # HAM warmup: the first ~11us are input-DMA latency with
                    # the PE idle, so the HAM clock gate holds the PE at
                    # 1.2 GHz and the first ~9us of real matmuls run cold.
                    # ~64 throwaway identity transposes span the head,
                    # latch the gate warm (~3.4us of sustained activity)
                    # and keep it warm until the first projection lands.
                    for w in range(8):
                        wt = psumAT.tile([P, NH, P], BF16, tag="tpA", name=f"warm{w}")
                        for j in range(NH):
                            nc.tensor.transpose(wt[:, j, :], identb, identb)
                    warmed = True
                aff_tiles = {}
                for name, flag, ap in (
                    ("g1", affine1, g1),
                    ("b1", affine1, b1),
                    ("g2", affine2, g2),
                    ("b2", affine2, b2),
                ):
                    if flag:
                        t = operands.tile([P, H], BF16, tag=name, name=f"aff_{name}")
                        rep = bass.AP(
                            tensor=ap.tensor, offset=ap.offset, ap=[[0, P], ap.ap[1]]
                        )
                        nc.sync.dma_start(out=t, in_=rep)
                        aff_tiles[name] = t
                if not affine1:
                    # k side skips the LN apply: rstd_k/H comes from
                    # Rsqrt(H^2*var + H^2*eps) per stripe below.
                    pass
                else:
                    nc.vector.memset(rstdk_h, 1.0 / H)

                # The two trailing stripes are k-side: their cast-only
                # epilogue unblocks the lag-flushed transposes ~1.2us
                # sooner than a full q LayerNorm chain would. (Interleaving
                # q stripes among the k block was tried and regressed ~6us.)
                mats = (
                    [("k", s) for s in range(NS - 2)]
                    + [("q", s) for s in range(NS)]
                    + [("k", NS - 2), ("k", NS - 1)]
                )

                def emit_transposes(which, s, nat):
                    """PE transposes + PSUM->SBUF copies for one stripe.

                    Emitted with a 3-stripe lag behind the projection
                    matmuls: the PE engine queue is strict FIFO, so a
                    transpose waiting on the LN epilogue would stall the
                    next stripe's matmuls; the resulting sub-3.5us idle
                    gaps kept the HAM clock gate oscillating and ran most
                    of phase A at 1.2 GHz.
                    """
                    sblk = bass.ts(s, P)
                    dstT = qT if which == "q" else kT
                    # All 8 transposed blocks land in ONE PSUM tile (8x128
                    # bf16 = exactly one 2KB bank) drained by ONE ACT copy:
                    # every ACT/DVE op pays a ~352-cycle startup, so one
                    # 1024-element copy beats two 512-element ones by ~300ns
                    # and keeps the whole drain off the busier DVE.
                    tp = psumAT.tile([P, NH, P], BF16, tag="tpA", name=f"tp_{which}{s}")
                    for j in range(NH):
                        nc.tensor.transpose(
                            tp[:, j, :],
                            nat[:, j * P : (j + 1) * P],
                            identb,
                        )
                    nc.scalar.copy(dstT[:, :, sblk], tp)

                pending = []
                for mi, (which, s) in enumerate(mats):
                    sblk = bass.ts(s, P)
                    lhs_all = xt_sb if which == "q" else yt_sb
                    rhs_all = q_sb if which == "q" else k_sb
                    # 3 rotating PSUM bank sets so the LayerNorm stats chain
                    # of stripe i drains while stripes i+1, i+2 accumulate.
                    pset = mi % 3
                    ps = psumA.tile([P, H], F32, tag=f"p{pset}", name=f"ps_{mi}")
                    for i in range(NF // 2):
                        for c in range(H // NC):
                            nc.tensor.matmul(
                                ps[:, c * NC : (c + 1) * NC],
                                lhs_all[:, 2 * i : 2 * i + 2, sblk],
                                rhs_all[:, 2 * i : 2 * i + 2, c * NC : (c + 1) * NC],
                                perf_mode=DR,
                                start=(i == 0),
                                stop=(i == NF // 2 - 1),
                            )
                    nat = work.tile([P, H], BF16, tag=f"{which}_nat")
                    st = stats_pool.tile([P, 2, 6], F32, tag="bn")
                    mv = stats_pool.tile([P, 2], F32, tag="mv")
                    if which == "k" and not affine1:
                        # Raw k is copied out first and the stats read the
                        # bf16 copy; the apply is skipped entirely:
                        # rstd_k/H = 1/sqrt(H^2*(var+eps)). (Computing the
                        # stats via ACT accum_out instead costs more: every
                        # ACT op pays a ~352-cycle startup, so 4 small ACT
                        # passes per stripe lose to one DVE bn_stats pair.)
                        nc.vector.tensor_copy(nat[:, 0:NC], ps[:, 0:NC])
                        nc.scalar.copy(nat[:, NC : 2 * NC], ps[:, NC : 2 * NC])
                        for i in range(2):
                            nc.vector.bn_stats(
                                out=st[:, i, :], in_=nat[:, i * NC : (i + 1) * NC]
                            )
                        nc.vector.bn_aggr(out=mv, in_=st)
                        sd = stats_pool.tile([P, 1], F32, tag="sd")
                        nc.scalar.activation(
                            out=sd,
                            in_=mv[:, 1:2],
                            func=AF.Sqrt,
                            bias=eps_h2,
                            scale=float(H * H),
                        )
                        nc.vector.reciprocal(out=rstdk_h[:, s : s + 1], in_=sd)
                    else:
                        # bn_stats free-dim limit is 512.
                        for i in range(2):
                            nc.vector.bn_stats(
                                out=st[:, i, :], in_=ps[:, i * NC : (i + 1) * NC]
                            )
                        nc.vector.bn_aggr(out=mv, in_=st)
                        rstd = stats_pool.tile([P, 1], F32, tag="rstd")
                        nc.scalar.activation(
                            out=rstd, in_=mv[:, 1:2], func=AF.Sqrt, bias=eps_sb
                        )
                        nc.vector.reciprocal(out=rstd, in_=rstd)
                        nbias = stats_pool.tile([P, 1], F32, tag="nbias")
                        nc.vector.tensor_scalar(
                            out=nbias,
                            in0=mv[:, 0:1],
                            scalar1=rstd,
                            scalar2=-1.0,
                            op0=mybir.AluOpType.mult,
                            op1=mybir.AluOpType.mult,
                        )
                        nc.vector.tensor_scalar(
                            out=nat[:, 0:NC],
                            in0=ps[:, 0:NC],
                            scalar1=mv[:, 0:1],
                            scalar2=rstd,
                            op0=mybir.AluOpType.subtract,
                            op1=mybir.AluOpType.mult,
                        )
                        nc.scalar.activation(
                            out=nat[:, NC : 2 * NC],
                            in_=ps[:, NC : 2 * NC],
                            func=AF.Identity,
                            bias=nbias,
                            scale=rstd,
                        )
                        gamma = aff_tiles.get("g2" if which == "q" else "g1")
                        beta = aff_tiles.get("b2" if which == "q" else "b1")
                        if gamma is not None:
                            nc.vector.tensor_mul(nat, nat, gamma)
                        if beta is not None:
                            nc.vector.tensor_add(nat, nat, beta)
                    pending.append((which, s, nat))
                    if len(pending) > 3:
                        emit_transposes(*pending.pop(0))
                for args in pending:
                    emit_transposes(*args)
                # Phase C operands: issued after the phase-A loads in trace
                # order so they don't delay the first matmuls.
                nc.sync.dma_start(
                    out=y_sb, in_=y8.rearrange("p (sb f) -> p sb f", sb=NS)
                )
                nc.sync.dma_start(out=crep, in_=cs)

            # ---- Phase B: transposed logits lT[t, q] per t-stripe ----
            # ---- Phase C: U[q, F] + denominator from the ones column ----
            with (
                tc.tile_pool(name="workBC", bufs=3) as workBC,
                tc.tile_pool(name="psumB", bufs=1, space="PSUM") as psumB,
                tc.tile_pool(name="psumC", bufs=2, space="PSUM") as psumC,
                tc.tile_pool(name="psumD", bufs=1, space="PSUM") as psumD,
            ):
                for st_i in range(NS):
                    tblk = bass.ts(st_i, P)
                    for c in range(S // NC):
                        lp = psumB.tile(
                            [P, NC],
                            F32,
                            tag=f"lp{(4 * st_i + c) % 3}",
                            name=f"lp{st_i}_{c}",
                        )
                        for g in range(NH // 2):
                            nc.tensor.matmul(
                                lp,
                                kT[:, 2 * g : 2 * g + 2, tblk],
                                qT[:, 2 * g : 2 * g + 2, c * NC : (c + 1) * NC],
                                perf_mode=DR,
                                start=(g == 0),
                                stop=(g == NH // 2 - 1),
                            )
                        ltmp = workBC.tile([P, NC], BF16, tag="ltmp")
                        nc.scalar.activation(
                            out=ltmp,
                            in_=lp,
                            func=AF.Exp,
                            scale=rstdk_h[:, st_i : st_i + 1],
                        )
                        # Delta softmax: exp(l)-1 applied during the fp8
                        # cast; the exact colsum(Y) is added back in phase C.
                        nc.vector.tensor_scalar_add(
                            expT8[:, st_i, c * NC : (c + 1) * NC], ltmp, -1.0
                        )

                for sq in range(NS):
                    sqblk = bass.ts(sq, P)
                    upd = psumC.tile([P, 2, NC], F32, tag="upd", name=f"upd{sq}")
                    dnp = psumD.tile([P, 1], F32, tag="dnp", name=f"dnp{sq}")
                    for j in range(NS // 2):
                        for c in range(F // NC):
                            nc.tensor.matmul(
                                upd[:, c, :],
                                expT8[:, 2 * j : 2 * j + 2, sqblk],
                                y_sb[:, 2 * j : 2 * j + 2, c * NC : (c + 1) * NC],
                                perf_mode=DR,
                                start=(j == 0),
                                stop=(j == NS // 2 - 1),
                            )
                            if c == 0:
                                # Denominator column rides mid-pass so its
                                # (redundant) LDWEIGHTS hides under chunk 0
                                # and chunk 1's hides under the den matmul.
                                nc.tensor.matmul(
                                    dnp,
                                    expT8[:, 2 * j : 2 * j + 2, sqblk],
                                    y_sb[:, 2 * j : 2 * j + 2, F : F + 1],
                                    perf_mode=DR,
                                    start=(j == 0),
                                    stop=(j == NS // 2 - 1),
                                )
                    den = stats_pool.tile([P, 1], F32, tag="den")
                    nc.vector.tensor_scalar_add(den, dnp, float(S))
                    nc.vector.reciprocal(out=recips[:, sq : sq + 1], in_=den)
                    o_st = workBC.tile([P, F], F32, tag="o_st")
                    for c in range(F // NC):
                        nc.vector.tensor_add(
                            upd[:, c, :], upd[:, c, :], crep[:, c * NC : (c + 1) * NC]
                        )
                        nc.scalar.activation(
                            out=o_st[:, c * NC : (c + 1) * NC],
                            in_=upd[:, c, :],
                            func=AF.Copy,
                            scale=recips[:, sq : sq + 1],
                        )
                    nc.sync.dma_start(
                        out=out[sq * P : (sq + 1) * P, :], in_=o_st
                    )

    nc.finalize()
    return nc


_NC_CACHE: dict = {}


def kernel(X, Y, K, Q, g1, b1, g2, b2, _trace=False, _trace_kwargs=None):
    B = X.shape[0]
    assert X.shape == (B, S, F) and Y.shape == (B, S, F)
    bf = ml_dtypes.bfloat16
    f8 = ml_dtypes.float8_e4m3

    affine1 = not (np.all(g1 == 1.0) and np.all(b1 == 0.0))
    affine2 = not (np.all(g2 == 1.0) and np.all(b2 == 0.0))

    key = (affine1, affine2)
    if key not in _NC_CACHE:
        _NC_CACHE[key] = _build_nc(affine1, affine2)
    nc = _NC_CACHE[key]

    def pack_rows(a, nb):
        """[nb*P, W] -> [P, nb*W] with row fb*P+p at [p, fb*W:(fb+1)*W]."""
        w = a.shape[1]
        return np.ascontiguousarray(
            a.reshape(nb, P, w).transpose(1, 0, 2).reshape(P, nb * w)
        )

    kw_b = pack_rows(np.ascontiguousarray(K).astype(f8), NF)
    qw_b = pack_rows(np.ascontiguousarray(Q).astype(f8), NF)
    in_maps = []
    for b in range(B):
        ya = np.zeros((S, FA), dtype=f8)
        ya[:, :F] = Y[b].astype(f8)
        ya[:, F] = 1.0
        m = {
            "XT": pack_rows(np.ascontiguousarray(X[b].T).astype(f8), NF),
            "YT": pack_rows(np.ascontiguousarray(Y[b].T).astype(f8), NF),
            "Y8": pack_rows(ya, NS),
            "CS": np.broadcast_to(
                Y[b].astype(np.float32).sum(0, keepdims=True), (P, F)
            ).copy(),
            "Kw": kw_b,
            "Qw": qw_b,
        }
        if affine1:
            m["g1r"] = g1.astype(bf).reshape(1, H)
            m["b1r"] = b1.astype(bf).reshape(1, H)
        if affine2:
            m["g2r"] = g2.astype(bf).reshape(1, H)
            m["b2r"] = b2.astype(bf).reshape(1, H)
        in_maps.append(m)

    res = run_bass_kernel_spmd(
        nc,
        in_maps,
        core_ids=list(range(B)),
        trace=_trace,
        **(_trace_kwargs or {}),
    )
    kernel.last_result = res
    return np.stack([r["out"] for r in res.results], axis=0).astype(np.float32)


# revision 30
# speedup vs baseline: 1.0733x; 1.0733x over previous
"""Trainium2 Bass kernel for batched attention with LayerNorm'd projections.

Reference computation (per batch element b):
    keys    = LN(Y[b] @ K) * g1 + b1          [S, H]
    queries = LN(X[b] @ Q) * g2 + b2          [S, H]
    alpha   = softmax(queries @ keys.T / H)   [S, S]
    out[b]  = alpha @ Y[b]                    [S, F]

Shapes: B=8, S=2048, F=H=1024. Data-parallel: one batch element per
NeuronCore, 8 cores, no collectives. Measured ~242us on silicon at
~4.2e-3 relative error (prior q-major-logits version: ~257us).

Device pipeline per core (transposed-logits orientation):
  A: q/k projections in natural [S,H] layout from fp8 XT/YT (contraction
     F on partitions, DoubleRow). LayerNorm stats via bn_stats on PSUM
     with 3 rotating 2-bank PSUM sets. The q side applies full LN
     ((x-mu)*rstd, split DVE/ACT) because the logits matmul needs a
     zero-mean q operand; the k side skips the apply entirely — raw k is
     transposed and rstd_k is folded into the exp scale later, and the
     k-mean term cancels exactly against zero-mean q̂:
        logit[t,q] = rstd_k[t]/H * sum_h k_raw[h,t] * qhat[h,q]
     PE 128x128 transposes move q̂/k_raw into H-major fp8 qT/kT with
     3072B row stride (2048B pair stride on the *streamed* operand
     halves the DoubleRow rate).
  B: logits computed TRANSPOSED, lT[t,q] = kT_block^T @ qT, per t-stripe
     [128, 2048]. ACT applies exp with per-partition scale rstd_k[t]/H;
     DVE applies exp(l)-1 during the fp8 cast into expT8 [t, q] (delta
     softmax: values ~±0.2 quantize ~20x better than ~1.0). Because
     expT8 is already t-major, NO alpha transposes are needed for the
     value matmul (the old orientation burned 256 PE transposes here).
  C: U stripe [q,F] = expT8_slice^T @ Y_aug where Y_aug carries a ones
     column: the same accumulation group yields the softmax denominator
     column den[q]-S for free. Exact f32 colsum(Y) (host computed) is
     added by DVE; the mandatory PSUM->SBUF copy applies 1/den.

Hardware constraints that shaped this:
  - fp32 matmul is 4x slower than bf16; fp8 DoubleRow is ~2x faster.
    Everything matmul-facing is fp8 with f32 PSUM accumulation.
  - PSUM is 8 banks: phase A uses 3x2 (projections) + 2 (transposes);
    phases B/C use 3 rotating logits banks + 2x2 output + 1 denominator.
  - The PE engine queue is strict FIFO and the HAM clock gate re-throttles
    the PE to 1.2 GHz after idle gaps: phase-A transposes are emitted with
    a 2-stripe lag behind the projection matmuls, and the 3 logits banks
    rotate globally (not per-stripe), so the PE never waits on a vector-
    engine epilogue. Before these two fixes ~15% of matmuls ran cold.
  - The first ~6us are fixed framework preamble (semaphore init + engine
    table loads) and the first projection operands take ~12us to land
    (per-queue DMA rate ~25-30GB/s); ~64 throwaway identity transposes
    fill that window so the HAM gate is already warm at first real work.
  - Run-to-run variance: under sustained load the chip drops to ~2.0 GHz
    (P0 power state), inflating the whole kernel ~20%; compare timings
    via the warm p50 matmul duration (215ns at 2.4 GHz, 259ns at 2.0).
"""

import numpy as np
import ml_dtypes

import concourse.bass as bass
import concourse.bacc as bacc
import concourse.tile as tile
from concourse import mybir
from concourse.bass_utils import run_bass_kernel_spmd
from concourse.masks import make_identity

BF16 = mybir.dt.bfloat16
FP8 = mybir.dt.float8e4
F32 = mybir.dt.float32
AF = mybir.ActivationFunctionType

S = 2048  # sequence length per core
SP = 3072  # padded qT/kT row stride (odd multiple of 1KB: avoids SBUF bank conflicts in DoubleRow pair fetch)
F = 1024  # input feature dim
FA = F + 16  # Y row padded with a ones column at index F (denominator trick)
H = 1024  # hidden dim
P = 128  # partitions
NS = S // P  # 16 sequence stripes
NF = F // P  # 8 contraction tiles for projections
NH = H // P  # 8 hidden tiles
NC = 512  # matmul free-dim chunk (one PSUM bank)
EPS = 1e-5


def _build_nc(affine1: bool, affine2: bool) -> bass.Bass:
    nc = bacc.Bacc(None)

    # All operands host-packed so each SBUF partition's data is one
    # contiguous DRAM run (2KB+ descriptor lines; strided layouts move
    # ~1KB per ~67ns descriptor slot and halve effective DMA rate).
    xt = nc.declare_dram_parameter("XT", [P, NF * S], FP8, isOutput=False)[:]
    yt = nc.declare_dram_parameter("YT", [P, NF * S], FP8, isOutput=False)[:]
    y8 = nc.declare_dram_parameter("Y8", [P, NS * FA], FP8, isOutput=False)[:]
    cs = nc.declare_dram_parameter("CS", [P, F], F32, isOutput=False)[:]
    kw = nc.declare_dram_parameter("Kw", [P, NF * H], FP8, isOutput=False)[:]
    qw = nc.declare_dram_parameter("Qw", [P, NF * H], FP8, isOutput=False)[:]
    g1 = b1 = g2 = b2 = None
    if affine1:
        g1 = nc.declare_dram_parameter("g1r", [1, H], BF16, isOutput=False)[:]
        b1 = nc.declare_dram_parameter("b1r", [1, H], BF16, isOutput=False)[:]
    if affine2:
        g2 = nc.declare_dram_parameter("g2r", [1, H], BF16, isOutput=False)[:]
        b2 = nc.declare_dram_parameter("b2r", [1, H], BF16, isOutput=False)[:]
    out = nc.declare_dram_parameter("out", [S, F], F32, isOutput=True)[:]

    with tile.TileContext(nc) as tc:
        with (
            tc.tile_pool(name="persist", bufs=1) as persist,
            tc.tile_pool(name="stats", bufs=8) as stats_pool,
        ):
            # Persistent SBUF tensors (whole-kernel lifetime).
            # Per-partition: qT 24k + kT 24k + expT8 32k + y_sb 16.25k +
            # crep 4k + small = ~101k.
            qT = persist.tile([P, NH, SP], FP8, tag="qT")  # q̂T [H, S+pad]
            kT = persist.tile([P, NH, SP], FP8, tag="kT")  # k_rawT [H, S+pad]
            # expT8 rows padded to the 3072B stride: it is the value matmul's
            # stationary operand, and LDWEIGHTS DoubleRow pair fetch at a
            # 2048B pair stride measured ~14ns/MM slower than at 3072B.
            expT8 = persist.tile([P, NS, SP], FP8, tag="expT8")  # expm1T [Sk, Sq]
            recips = persist.tile([P, NS], F32, tag="recips")
            rstdk_h = persist.tile([P, NS], F32, tag="rstdk_h")  # rstd_k/H per t
            y_sb = persist.tile([P, NS, FA], FP8, tag="y_sb")  # Y_aug [Sk, F+1s]
            crep = persist.tile([P, F], F32, tag="crep")  # colsum(Y) bcast
            eps_sb = persist.tile([P, 1], F32, tag="eps")
            nc.vector.memset(eps_sb, EPS)
            eps_h2 = persist.tile([P, 1], F32, tag="eps_h2")
            nc.vector.memset(eps_h2, EPS * H * H)
            identb = persist.tile([P, P], BF16, tag="identb")
            make_identity(nc, identb)
            warmed = False

            # ---- Phase A: projections + LN + transpose to H-major ----
            DR = mybir.MatmulPerfMode.DoubleRow
            with (
                tc.tile_pool(name="operands", bufs=1) as operands,
                tc.tile_pool(name="work", bufs=3) as work,
                tc.tile_pool(name="psumA", bufs=1, space="PSUM") as psumA,
                tc.tile_pool(name="psumAT", bufs=2, space="PSUM") as psumAT,
            ):
                # All projection operands SBUF-resident in fp8:
                # xt/yt 16k + q/k 8k each = 48k per partition.
                xt_sb = operands.tile([P, NF, S], FP8, tag="xt_sb")
                yt_sb = operands.tile([P, NF, S], FP8, tag="yt_sb")
                q_sb = operands.tile([P, NF, H], FP8, tag="q_sb")
                k_sb = operands.tile([P, NF, H], FP8, tag="k_sb")
                xt_r = xt.rearrange("p (fb s) -> p fb s", fb=NF)
                yt_r = yt.rearrange("p (fb s) -> p fb s", fb=NF)
                qw_r = qw.rearrange("p (fb h) -> p fb h", fb=NF)
                kw_r = kw.rearrange("p (fb h) -> p fb h", fb=NF)
                # Load order tuned for earliest first-stripe start: the
                # first k-projection stripes need k pairs (full) and the
                # low column quarter of every yt pair. Whole k pairs are
                # single DMAs (contiguous 2KB/partition runs thanks to
                # the host packing); yt/xt are column-split since stripes
                # consume them 128 columns at a time.
                for i in range(NF // 2):
                    # Partition-halved DMAs keep the contiguous 2KB lines
                    # while landing each piece on its own queue, halving
                    # the time to the first stripe's operands.
                    for plo, phi in ((0, P // 2), (P // 2, P)):
                        nc.sync.dma_start(
                            out=k_sb[plo:phi, 2 * i : 2 * i + 2, :],
                            in_=kw_r[plo:phi, 2 * i : 2 * i + 2, :],
                        )
                        nc.sync.dma_start(
                            out=yt_sb[plo:phi, 2 * i : 2 * i + 2, 0 : S // 2],
                            in_=yt_r[plo:phi, 2 * i : 2 * i + 2, 0 : S // 2],
                        )
                for i in range(NF // 2):
                    nc.sync.dma_start(
                        out=yt_sb[:, 2 * i : 2 * i + 2, S // 2 : S],
                        in_=yt_r[:, 2 * i : 2 * i + 2, S // 2 : S],
                    )
                for i in range(NF // 2):
                    nc.sync.dma_start(
                        out=q_sb[:, 2 * i : 2 * i + 2, :],
                        in_=qw_r[:, 2 * i : 2 * i + 2, :],
                    )
                    nc.sync.dma_start(
                        out=xt_sb[:, 2 * i : 2 * i + 2, 0 : S // 2],
                        in_=xt_r[:, 2 * i : 2 * i + 2, 0 : S // 2],
                    )
                for i in range(NF // 2):
                    nc.sync.dma_start(
                        out=xt_sb[:, 2 * i : 2 * i + 2, S // 2 : S],
                        in_=xt_r[:, 2 * i : 2 * i + 2, S // 2 : S],
                    )
                if not warmed:
                    # HAM warmup: the first ~11us are input-DMA latency with
                    # the PE idle, so the HAM clock gate holds the PE at
                    # 1.2 GHz and the first ~9us of real matmuls run cold.
                    # ~64 throwaway identity transposes span the head,
                    # latch the gate warm (~3.4us of sustained activity)
                    # and keep it warm until the first projection lands.
                    for w in range(8):
                        wt = psumAT.tile([P, NH, P], BF16, tag="tpA", name=f"warm{w}")
                        for j in range(NH):
                            nc.tensor.transpose(wt[:, j, :], identb, identb)
                    warmed = True
                aff_tiles = {}
                for name, flag, ap in (
                    ("g1", affine1, g1),
                    ("b1", affine1, b1),
                    ("g2", affine2, g2),
                    ("b2", affine2, b2),
                ):
                    if flag:
                        t = operands.tile([P, H], BF16, tag=name, name=f"aff_{name}")
                        rep = bass.AP(
                            tensor=ap.tensor, offset=ap.offset, ap=[[0, P], ap.ap[1]]
                        )
                        nc.sync.dma_start(out=t, in_=rep)
                        aff_tiles[name] = t
                if not affine1:
                    # k side skips the LN apply: rstd_k/H comes from
                    # Rsqrt(H^2*var + H^2*eps) per stripe below.
                    pass
                else:
                    nc.vector.memset(rstdk_h, 1.0 / H)

                # The two trailing stripes are k-side: their cast-only
                # epilogue unblocks the lag-flushed transposes ~1.2us
                # sooner than a full q LayerNorm chain would. (Interleaving
                # q stripes among the k block was tried and regressed ~6us.)
                mats = (
                    [("k", s) for s in range(NS - 2)]
                    + [("q", s) for s in range(NS)]
                    + [("k", NS - 2), ("k", NS - 1)]
                )

                def emit_transposes(which, s, nat):
                    """PE transposes + PSUM->SBUF copies for one stripe.

                    Emitted with a 2-stripe lag behind the projection
                    matmuls: the PE engine queue is strict FIFO, so a
                    transpose waiting on the LN epilogue would stall the
                    next stripe's matmuls; the resulting sub-3.5us idle
                    gaps kept the HAM clock gate oscillating and ran most
                    of phase A at 1.2 GHz.
                    """
                    sblk = bass.ts(s, P)
                    dstT = qT if which == "q" else kT
                    # All 8 transposed blocks land in ONE PSUM tile (8x128
                    # bf16 = exactly one 2KB bank) drained by ONE ACT copy:
                    # every ACT/DVE op pays a ~352-cycle startup, so one
                    # 1024-element copy beats two 512-element ones by ~300ns
                    # and keeps the whole drain off the busier DVE.
                    tp = psumAT.tile([P, NH, P], BF16, tag="tpA", name=f"tp_{which}{s}")
                    for j in range(NH):
                        nc.tensor.transpose(
                            tp[:, j, :],
                            nat[:, j * P : (j + 1) * P],
                            identb,
                        )
                    nc.scalar.copy(dstT[:, :, sblk], tp)

                pending = []
                for mi, (which, s) in enumerate(mats):
                    sblk = bass.ts(s, P)
                    lhs_all = xt_sb if which == "q" else yt_sb
                    rhs_all = q_sb if which == "q" else k_sb
                    # 3 rotating PSUM bank sets so the LayerNorm stats chain
                    # of stripe i drains while stripes i+1, i+2 accumulate.
                    pset = mi % 3
                    ps = psumA.tile([P, H], F32, tag=f"p{pset}", name=f"ps_{mi}")
                    for i in range(NF // 2):
                        for c in range(H // NC):
                            nc.tensor.matmul(
                                ps[:, c * NC : (c + 1) * NC],
                                lhs_all[:, 2 * i : 2 * i + 2, sblk],
                                rhs_all[:, 2 * i : 2 * i + 2, c * NC : (c + 1) * NC],
                                perf_mode=DR,
                                start=(i == 0),
                                stop=(i == NF // 2 - 1),
                            )
                    nat = work.tile([P, H], BF16, tag=f"{which}_nat")
                    st = stats_pool.tile([P, 2, 6], F32, tag="bn")
                    mv = stats_pool.tile([P, 2], F32, tag="mv")
                    if which == "k" and not affine1:
                        # Raw k is copied out first and the stats read the
                        # bf16 copy; the apply is skipped entirely:
                        # rstd_k/H = 1/sqrt(H^2*(var+eps)). (Computing the
                        # stats via ACT accum_out instead costs more: every
                        # ACT op pays a ~352-cycle startup, so 4 small ACT
                        # passes per stripe lose to one DVE bn_stats pair.)
                        nc.vector.tensor_copy(nat[:, 0:NC], ps[:, 0:NC])
                        nc.scalar.copy(nat[:, NC : 2 * NC], ps[:, NC : 2 * NC])
                        for i in range(2):
                            nc.vector.bn_stats(
                                out=st[:, i, :], in_=nat[:, i * NC : (i + 1) * NC]
                            )
                        nc.vector.bn_aggr(out=mv, in_=st)
                        sd = stats_pool.tile([P, 1], F32, tag="sd")
                        nc.scalar.activation(
                            out=sd,
                            in_=mv[:, 1:2],
                            func=AF.Sqrt,
                            bias=eps_h2,
                            scale=float(H * H),
                        )
                        nc.vector.reciprocal(out=rstdk_h[:, s : s + 1], in_=sd)
                    else:
                        # bn_stats free-dim limit is 512.
                        for i in range(2):
                            nc.vector.bn_stats(
                                out=st[:, i, :], in_=ps[:, i * NC : (i + 1) * NC]
                            )
                        nc.vector.bn_aggr(out=mv, in_=st)
                        rstd = stats_pool.tile([P, 1], F32, tag="rstd")
                        nc.scalar.activation(
                            out=rstd, in_=mv[:, 1:2], func=AF.Sqrt, bias=eps_sb
                        )
                        nc.vector.reciprocal(out=rstd, in_=rstd)
                        nbias = stats_pool.tile([P, 1], F32, tag="nbias")
                        nc.vector.tensor_scalar(
                            out=nbias,
                            in0=mv[:, 0:1],
                            scalar1=rstd,
                            scalar2=-1.0,
                            op0=mybir.AluOpType.mult,
                            op1=mybir.AluOpType.mult,
                        )
                        nc.vector.tensor_scalar(
                            out=nat[:, 0:NC],
                            in0=ps[:, 0:NC],
                            scalar1=mv[:, 0:1],
                            scalar2=rstd,
                            op0=mybir.AluOpType.subtract,
                            op1=mybir.AluOpType.mult,
                        )
                        nc.scalar.activation(
                            out=nat[:, NC : 2 * NC],
                            in_=ps[:, NC : 2 * NC],
                            func=AF.Identity,
                            bias=nbias,
                            scale=rstd,
                        )
                        gamma = aff_tiles.get("g2" if which == "q" else "g1")
                        beta = aff_tiles.get("b2" if which == "q" else "b1")
                        if gamma is not None:
                            nc.vector.tensor_mul(nat, nat, gamma)
                        if beta is not None:
                            nc.vector.tensor_add(nat, nat, beta)
                    pending.append((which, s, nat))
                    if len(pending) > 2:
                        emit_transposes(*pending.pop(0))
                for args in pending:
                    emit_transposes(*args)
                # Phase C operands: issued after the phase-A loads in trace
                # order so they don't delay the first matmuls.
                nc.sync.dma_start(
                    out=y_sb, in_=y8.rearrange("p (sb f) -> p sb f", sb=NS)
                )
                nc.sync.dma_start(out=crep, in_=cs)

            # ---- Phase B: transposed logits lT[t, q] per t-stripe ----
            # ---- Phase C: U[q, F] + denominator from the ones column ----
            with (
                tc.tile_pool(name="workBC", bufs=3) as workBC,
                tc.tile_pool(name="psumB", bufs=1, space="PSUM") as psumB,
                tc.tile_pool(name="psumC", bufs=2, space="PSUM") as psumC,
                tc.tile_pool(name="psumD", bufs=1, space="PSUM") as psumD,
            ):
                for st_i in range(NS):
                    tblk = bass.ts(st_i, P)
                    for c in range(S // NC):
                        lp = psumB.tile(
                            [P, NC],
                            F32,
                            tag=f"lp{(4 * st_i + c) % 3}",
                            name=f"lp{st_i}_{c}",
                        )
                        for g in range(NH // 2):
                            nc.tensor.matmul(
                                lp,
                                kT[:, 2 * g : 2 * g + 2, tblk],
                                qT[:, 2 * g : 2 * g + 2, c * NC : (c + 1) * NC],
                                perf_mode=DR,
                                start=(g == 0),
                                stop=(g == NH // 2 - 1),
                            )
                        ltmp = workBC.tile([P, NC], BF16, tag="ltmp")
                        nc.scalar.activation(
                            out=ltmp,
                            in_=lp,
                            func=AF.Exp,
                            scale=rstdk_h[:, st_i : st_i + 1],
                        )
                        # Delta softmax: exp(l)-1 applied during the fp8
                        # cast; the exact colsum(Y) is added back in phase C.
                        nc.vector.tensor_scalar_add(
                            expT8[:, st_i, c * NC : (c + 1) * NC], ltmp, -1.0
                        )

                for sq in range(NS):
                    sqblk = bass.ts(sq, P)
                    upd = psumC.tile([P, 2, NC], F32, tag="upd", name=f"upd{sq}")
                    dnp = psumD.tile([P, 1], F32, tag="dnp", name=f"dnp{sq}")
                    for j in range(NS // 2):
                        for c in range(F // NC):
                            nc.tensor.matmul(
                                upd[:, c, :],
                                expT8[:, 2 * j : 2 * j + 2, sqblk],
                                y_sb[:, 2 * j : 2 * j + 2, c * NC : (c + 1) * NC],
                                perf_mode=DR,
                                start=(j == 0),
                                stop=(j == NS // 2 - 1),
                            )
                            if c == 0:
                                # Denominator column rides mid-pass so its
                                # (redundant) LDWEIGHTS hides under chunk 0
                                # and chunk 1's hides under the den matmul.
                                nc.tensor.matmul(
                                    dnp,
                                    expT8[:, 2 * j : 2 * j + 2, sqblk],
                                    y_sb[:, 2 * j : 2 * j + 2, F : F + 1],
                                    perf_mode=DR,
                                    start=(j == 0),
                                    stop=(j == NS // 2 - 1),
                                )
                    den = stats_pool.tile([P, 1], F32, tag="den")
                    nc.vector.tensor_scalar_add(den, dnp, float(S))
                    nc.vector.reciprocal(out=recips[:, sq : sq + 1], in_=den)
                    o_st = workBC.tile([P, F], F32, tag="o_st")
                    for c in range(F // NC):
                        nc.vector.tensor_add(
                            upd[:, c, :], upd[:, c, :], crep[:, c * NC : (c + 1) * NC]
                        )
                        nc.scalar.activation(
                            out=o_st[:, c * NC : (c + 1) * NC],
                            in_=upd[:, c, :],
                            func=AF.Copy,
                            scale=recips[:, sq : sq + 1],
                        )
                    nc.sync.dma_start(
                        out=out[sq * P : (sq + 1) * P, :], in_=o_st
                    )

    nc.finalize()
    return nc


_NC_CACHE: dict = {}


def kernel(X, Y, K, Q, g1, b1, g2, b2, _trace=False, _trace_kwargs=None):
    B = X.shape[0]
    assert X.shape == (B, S, F) and Y.shape == (B, S, F)
    bf = ml_dtypes.bfloat16
    f8 = ml_dtypes.float8_e4m3

    affine1 = not (np.all(g1 == 1.0) and np.all(b1 == 0.0))
    affine2 = not (np.all(g2 == 1.0) and np.all(b2 == 0.0))

    key = (affine1, affine2)
    if key not in _NC_CACHE:
        _NC_CACHE[key] = _build_nc(affine1, affine2)
    nc = _NC_CACHE[key]

    def pack_rows(a, nb):
        """[nb*P, W] -> [P, nb*W] with row fb*P+p at [p, fb*W:(fb+1)*W]."""
        w = a.shape[1]
        return np.ascontiguousarray(
            a.reshape(nb, P, w).transpose(1, 0, 2).reshape(P, nb * w)
        )

    kw_b = pack_rows(np.ascontiguousarray(K).astype(f8), NF)
    qw_b = pack_rows(np.ascontiguousarray(Q).astype(f8), NF)
    in_maps = []
    for b in range(B):
        ya = np.zeros((S, FA), dtype=f8)
        ya[:, :F] = Y[b].astype(f8)
        ya[:, F] = 1.0
        m = {
            "XT": pack_rows(np.ascontiguousarray(X[b].T).astype(f8), NF),
            "YT": pack_rows(np.ascontiguousarray(Y[b].T).astype(f8), NF),
            "Y8": pack_rows(ya, NS),
            "CS": np.broadcast_to(
                Y[b].astype(np.float32).sum(0, keepdims=True), (P, F)
            ).copy(),
            "Kw": kw_b,
            "Qw": qw_b,
        }
        if affine1:
            m["g1r"] = g1.astype(bf).reshape(1, H)
            m["b1r"] = b1.astype(bf).reshape(1, H)
        if affine2:
            m["g2r"] = g2.astype(bf).reshape(1, H)
            m["b2r"] = b2.astype(bf).reshape(1, H)
        in_maps.append(m)

    res = run_bass_kernel_spmd(
        nc,
        in_maps,
        core_ids=list(range(B)),
        trace=_trace,
        **(_trace_kwargs or {}),
    )
    kernel.last_result = res
    return np.stack([r["out"] for r in res.results], axis=0).astype(np.float32)
